# revision 46
# baseline (speedup 1.0000x reference)
"""Trainium2 Bass kernel for nn_Decoder (sparse_attention, B=512, N=1000, D=128, H=8).

Strategy
--------
Pure data parallel: batch B=512 sharded across 8 NeuronCores (64 per core).

The reference computes, per batch b:
    gc      = mean_n(emb) @ W_fixed                          [D]
    q       = gc + ctx @ W_step                              [D]
    K,V,lk  = heads(emb @ W_node)                            (never materialized here)
    compat  = (q_h . K_h[n]) / sqrt(dk)                      [H,N]
    attn    = softmax(mask(compat))                          [H,N]
    glimpse = concat_h(attn_h @ V_h) @ W_out                 [D]
    logits  = mask(tanh(glimpse . lk[n] / sqrt(D)) * 10)     [N]

We fold the big [D,3D] projection into per-batch vectors so embeddings are the
only large tensor ever touched:
    compat[n,h] = emb[n,:] @ U[:,h]    with  U = W_gk @ blockdiag(q)/sqrt(dk)
    aT[:,h]     = sum_n attn[h,n] emb[n,:]   (attention-weighted embedding sum)
    glimpse     = (W_out.T @ select_heads(W_gv.T @ aT)),  w_l = W_lk @ glimpse/sqrt(D)
    logits[n]   = emb[n,:] @ w_l

Softmax is computed without max-subtraction (compat range is ~[-10, 10] for
these inputs, exp is safe in fp32) and masking is applied multiplicatively
after exp, which is mathematically identical to -inf masking.

Per core, batches are processed in 4 groups of 16 so all tail element-wise ops
run on fully packed [128, *] tiles. Embeddings are supplied in two bf16
layouts (n-major and d-major, host-prepared) because the TensorEngine can only
contract over the partition axis; all PE passes then stream 8-16 column moving
operands against 128x128 embedding chunks loaded as (fast-weight-load) weights.
"""

import numpy as np
import ml_dtypes

import concourse.bass as bass
from concourse import bacc
from concourse.tile import TileContext
from concourse import mybir
from concourse.bass_utils import run_bass_kernel_spmd

F32 = mybir.dt.float32
BF16 = mybir.dt.bfloat16
BF16_NP = ml_dtypes.bfloat16

B, N, D, H = 512, 1000, 128, 8
DK = D // H
NCORES = 8
BC = B // NCORES          # 64 batches per core
NP = 1024                 # padded N
CH = NP // 128            # 8 chunks of 128 nodes
G = 16                    # batches per group
NG = BC // G              # groups per core
CF = G * CH * H           # compat/attn free size per group
LF = G * CH               # logits free size per group
UF = G * H                # (batch, head) packed free size

_NC_CACHE = None


def _build_nc():
    """Build the single-core Bass program (shared SPMD across 8 cores)."""
    nc = bacc.Bacc("TRN2", target_bir_lowering=False)

    # partition-major group layout: [g][p][b within group][n] — per-partition
    # contiguous 32 KB per group for full-rate DMA
    e_nd_d = nc.dram_tensor("e_nd", [NG, 128, G, NP], BF16, kind="ExternalInput")
    e_dn_d = nc.dram_tensor("e_dn", [NG, 128, G, NP], BF16, kind="ExternalInput")
    feas_d = nc.dram_tensor("feas", [NG, 128, CF], BF16, kind="ExternalInput")
    binf_d = nc.dram_tensor("binf", [NG, 128, LF], F32, kind="ExternalInput")
    ctxT_d = nc.dram_tensor("ctxT", [2, 128, BC], F32, kind="ExternalInput")
    wfix_d = nc.dram_tensor("wfix", [128, 128], F32, kind="ExternalInput")
    wstep_d = nc.dram_tensor("wstep", [2, 128, 128], F32, kind="ExternalInput")
    wgkT_d = nc.dram_tensor("wgkT", [128, 128], F32, kind="ExternalInput")
    wgv_d = nc.dram_tensor("wgv", [128, 128], BF16, kind="ExternalInput")
    wout_d = nc.dram_tensor("wout", [128, 128], BF16, kind="ExternalInput")
    wlkT_d = nc.dram_tensor("wlkT", [128, 128], BF16, kind="ExternalInput")
    hmask_d = nc.dram_tensor("hmask", [128, UF], F32, kind="ExternalInput")
    out_d = nc.dram_tensor("out_dev", [NG, 128, LF], F32, kind="ExternalOutput")

    with TileContext(nc) as tc:
        with (
            tc.tile_pool(name="consts", bufs=1) as consts,
            tc.tile_pool(name="emb", bufs=5) as epool,
            tc.tile_pool(name="grp", bufs=2) as gpool,
            tc.tile_pool(name="scr", bufs=2) as spool,
            tc.tile_pool(name="pbig", bufs=2, space="PSUM") as pbig,
            tc.tile_pool(name="pat", bufs=1, space="PSUM") as pat,
            tc.tile_pool(name="pmisc", bufs=3, space="PSUM") as pmisc,
        ):
            # ---- constants ----
            wfix_sb = consts.tile([128, 128], F32)
            nc.sync.dma_start(out=wfix_sb, in_=wfix_d[:, :])
            wstep_sb = consts.tile([128, 2, 128], F32)
            nc.sync.dma_start(out=wstep_sb, in_=wstep_d.rearrange("k p j -> p k j"))
            wgkT_sb = consts.tile([128, 128], F32)
            nc.sync.dma_start(out=wgkT_sb, in_=wgkT_d[:, :])
            wgv_sb = consts.tile([128, 128], BF16)
            nc.sync.dma_start(out=wgv_sb, in_=wgv_d[:, :])
            wout_sb = consts.tile([128, 128], BF16)
            nc.sync.dma_start(out=wout_sb, in_=wout_d[:, :])
            wlkT_sb = consts.tile([128, 128], BF16)
            nc.sync.dma_start(out=wlkT_sb, in_=wlkT_d[:, :])
            hmask_sb = consts.tile([128, UF], F32)
            nc.sync.dma_start(out=hmask_sb, in_=hmask_d[:, :])
            ctxT_sb = consts.tile([128, 2, BC], F32)
            nc.sync.dma_start(out=ctxT_sb, in_=ctxT_d.rearrange("k p b -> p k b"))

            ones_col = consts.tile([128, 1], BF16)
            nc.vector.memset(ones_col, 1.0)
            ones_row = consts.tile([1, 128], F32)
            nc.vector.memset(ones_row, 1.0)

            # step_cols[j, b] = (ctx[b] @ W_step/4)[j]   (one-time)
            step_ps = pmisc.tile([128, BC], F32, tag="misc")
            for k in range(2):
                nc.tensor.matmul(
                    step_ps,
                    lhsT=wstep_sb[:, k, :],
                    rhs=ctxT_sb[:, k, :],
                    start=(k == 0),
                    stop=(k == 1),
                )
            step_cols = consts.tile([128, BC], F32)
            nc.vector.tensor_copy(step_cols, step_ps)

            # software-pipelined emission: group g+1's loads and mean
            # reductions are emitted during group g's tail so each engine's
            # (strictly ordered) instruction stream never head-of-line-blocks
            # the next group's prologue behind this group's tail.
            def load_group(g):
                HG = G // 2
                e_dn_h, e_nd_h = [], []
                for h in range(2):
                    t = epool.tile([128, HG, NP], BF16, tag="e_dn", name=f"edn{g}_{h}")
                    nc.sync.dma_start(out=t, in_=e_dn_d[g, :, h * HG : (h + 1) * HG, :])
                    e_dn_h.append(t)
                    t = epool.tile([128, HG, NP], BF16, tag="e_nd", name=f"end{g}_{h}")
                    nc.sync.dma_start(out=t, in_=e_nd_d[g, :, h * HG : (h + 1) * HG, :])
                    e_nd_h.append(t)
                e_dns = [e_dn_h[bb // HG][:, bb % HG, :] for bb in range(G)]
                e_nds = [e_nd_h[bb // HG][:, bb % HG, :] for bb in range(G)]
                return e_dns, e_nds

            def do_means(g, e_dns):
                # raw embedding sums (scale folded into wfix host-side),
                # split ACT/DVE so neither engine bottlenecks
                mean_cols = gpool.tile([128, G], F32, tag="mean", name=f"mc{g}")
                for bb in range(G):
                    if bb % 2 == 0:
                        scr = spool.tile([128, NP], BF16, tag="scr", name=f"scr{g}_{bb}")
                        nc.scalar.activation(
                            out=scr,
                            in_=e_dns[bb],
                            func=mybir.ActivationFunctionType.Copy,
                            bias=0.0,
                            scale=1.0,
                            accum_out=mean_cols[:, bb : bb + 1],
                        )
                    else:
                        nc.vector.tensor_reduce(
                            out=mean_cols[:, bb : bb + 1],
                            in_=e_dns[bb],
                            axis=mybir.AxisListType.X,
                            op=mybir.AluOpType.add,
                        )
                return mean_cols

            tiles = {0: load_group(0)}
            means = {0: do_means(0, tiles[0][0])}
            for g in range(NG):
                e_dns, e_nds = tiles.pop(g)
                mean_cols = means.pop(g)
                if g + 1 < NG:
                    tiles[g + 1] = load_group(g + 1)

                # ---- q = mean@W_fixed + step;  U = W_gk.T-combine(blockdiag q) ----
                q_ps = pmisc.tile([128, G], F32, tag="misc", name=f"qps{g}")
                nc.tensor.matmul(q_ps, lhsT=wfix_sb, rhs=mean_cols, start=True, stop=True)
                qs = gpool.tile([128, G], F32, tag="qs")
                nc.vector.tensor_tensor(
                    qs, q_ps, step_cols[:, g * G : (g + 1) * G], mybir.AluOpType.add
                )
                qblk = gpool.tile([128, UF], F32, tag="qblk")
                nc.vector.scalar_tensor_tensor(
                    out=qblk.rearrange("p (bb h) -> p bb h", bb=G),
                    in0=qs.unsqueeze(2).broadcast_to((128, G, H)),
                    scalar=1.0,
                    in1=hmask_sb.rearrange("p (bb h) -> p bb h", bb=G),
                    op0=mybir.AluOpType.mult,
                    op1=mybir.AluOpType.mult,
                )
                u_ps = pmisc.tile([128, UF], F32, tag="misc", name=f"ups{g}")
                nc.tensor.matmul(u_ps, lhsT=wgkT_sb, rhs=qblk, start=True, stop=True)
                u_sb = gpool.tile([128, UF], BF16, tag="u")
                nc.vector.tensor_copy(u_sb, u_ps)

                # ---- compat[n, (bb,c,h)] = emb_dn_chunk.T @ U[b] ----
                compat_ps = pbig.tile([128, CF], F32, tag="big", name=f"cp{g}")
                for bb in range(G):
                    for c in range(CH):
                        o = 64 * bb + 8 * c
                        nc.tensor.matmul(
                            compat_ps[:, o : o + 8],
                            lhsT=e_dns[bb][:, 128 * c : 128 * (c + 1)],
                            rhs=u_sb[:, 8 * bb : 8 * bb + 8],
                            start=True,
                            stop=True,
                        )

                # ---- attn = exp(compat) * feas   (no max-sub: |compat| < ~15) ----
                attn = gpool.tile([128, CF], BF16, tag="attn")
                nc.scalar.activation(
                    out=attn,
                    in_=compat_ps,
                    func=mybir.ActivationFunctionType.Exp,
                    bias=0.0,
                    scale=1.0,
                )
                feas_sb = gpool.tile([128, CF], BF16, tag="feas")
                nc.gpsimd.dma_start(out=feas_sb, in_=feas_d[g, :, :])
                nc.vector.tensor_tensor(attn, attn, feas_sb, mybir.AluOpType.mult)

                # ---- Z (per bb one matmul) and aT (8-chunk accumulation) ----
                z_ps = pbig.tile([1, CF], F32, tag="big", name=f"zp{g}")
                at_ps = pat.tile([128, UF], F32, tag="at", name=f"at{g}")
                aT_sb = gpool.tile([128, UF], BF16, tag="aT")
                for bb in range(G):
                    nc.tensor.matmul(
                        z_ps[:, 64 * bb : 64 * (bb + 1)],
                        lhsT=ones_col,
                        rhs=attn[:, 64 * bb : 64 * (bb + 1)],
                        start=True,
                        stop=True,
                    )
                    # accumulate into this batch's 8-col slice; PE executes its
                    # stream in order so per-batch accumulation groups in this
                    # bank never interleave (CoreSim's group-check verifies)
                    for c in range(CH):
                        nc.tensor.matmul(
                            at_ps[:, 8 * bb : 8 * bb + 8],
                            lhsT=e_nds[bb][:, 128 * c : 128 * (c + 1)],
                            rhs=attn[:, 64 * bb + 8 * c : 64 * bb + 8 * c + 8],
                            start=(c == 0),
                            stop=(c == CH - 1),
                        )
                nc.scalar.copy(aT_sb, at_ps)

                # Z reduce over chunks -> [1, (bb,h)], broadcast via outer product
                z_red = gpool.tile([1, UF], F32, tag="zred")
                nc.vector.tensor_reduce(
                    out=z_red.rearrange("p (bb h) -> p bb h", bb=G),
                    in_=z_ps.rearrange("p (bb c h) -> p bb h c", bb=G, c=CH),
                    axis=mybir.AxisListType.X,
                    op=mybir.AluOpType.add,
                )
                rz = gpool.tile([1, UF], F32, tag="rz")
                nc.vector.reciprocal(rz, z_red)
                zb_ps = pmisc.tile([128, UF], F32, tag="misc", name=f"zb{g}")
                nc.tensor.matmul(zb_ps, lhsT=ones_row, rhs=rz, start=True, stop=True)
                zb_sb = gpool.tile([128, UF], F32, tag="zb")
                nc.vector.tensor_copy(zb_sb, zb_ps)

                # ---- X = W_gv.T-path; normalize by Z; select head blocks ----
                x_ps = pmisc.tile([128, UF], F32, tag="misc", name=f"xps{g}")
                nc.tensor.matmul(x_ps, lhsT=wgv_sb, rhs=aT_sb, start=True, stop=True)
                xn = gpool.tile([128, UF], F32, tag="xn")
                nc.vector.tensor_tensor(xn, x_ps, zb_sb, mybir.AluOpType.mult)
                xh = gpool.tile([128, UF], F32, tag="xh")
                nc.vector.tensor_tensor(xh, xn, hmask_sb, mybir.AluOpType.mult)
                heads = gpool.tile([128, G], BF16, tag="heads")
                with nc.allow_low_precision(reason="8-wide head-select sum, bf16 out"):
                    nc.vector.tensor_reduce(
                        out=heads,
                        in_=xh.rearrange("p (bb h) -> p bb h", bb=G),
                        axis=mybir.AxisListType.X,
                        op=mybir.AluOpType.add,
                    )

                # ---- glimpse, w_l ----
                gl_ps = pmisc.tile([128, G], F32, tag="misc", name=f"gl{g}")
                nc.tensor.matmul(gl_ps, lhsT=wout_sb, rhs=heads, start=True, stop=True)
                gl_sb = gpool.tile([128, G], BF16, tag="gl")
                nc.vector.tensor_copy(gl_sb, gl_ps)
                wl_ps = pmisc.tile([128, G], F32, tag="misc", name=f"wl{g}")
                nc.tensor.matmul(wl_ps, lhsT=wlkT_sb, rhs=gl_sb, start=True, stop=True)
                wl_sb = gpool.tile([128, G], BF16, tag="wl")
                nc.vector.tensor_copy(wl_sb, wl_ps)

                # prefetch next group's mean reductions into the ACT/DVE
                # streams here — they execute while PE runs this group's logits
                if g + 1 < NG:
                    means[g + 1] = do_means(g + 1, tiles[g + 1][0])

                # ---- logits[n-in-chunk, (bb,c)] = emb_dn_chunk.T @ w_l[b] ----
                log_ps = pmisc.tile([128, LF], F32, tag="misc", name=f"lg{g}")
                for bb in range(G):
                    for c in range(CH):
                        nc.tensor.matmul(
                            log_ps[:, 8 * bb + c : 8 * bb + c + 1],
                            lhsT=e_dns[bb][:, 128 * c : 128 * (c + 1)],
                            rhs=wl_sb[:, bb : bb + 1],
                            start=True,
                            stop=True,
                        )

                # ---- tanh, *10, -inf mask, store ----
                th = gpool.tile([128, LF], F32, tag="th")
                nc.scalar.activation(
                    out=th,
                    in_=log_ps,
                    func=mybir.ActivationFunctionType.Tanh,
                    bias=0.0,
                    scale=1.0,
                )
                binf_sb = gpool.tile([128, LF], F32, tag="binf")
                nc.gpsimd.dma_start(out=binf_sb, in_=binf_d[g, :, :])
                out_sb = gpool.tile([128, LF], F32, tag="out")
                nc.vector.scalar_tensor_tensor(
                    out=out_sb,
                    in0=th,
                    scalar=10.0,
                    in1=binf_sb,
                    op0=mybir.AluOpType.mult,
                    op1=mybir.AluOpType.add,
                )
                nc.gpsimd.dma_start(out=out_d[g, :, :], in_=out_sb)

    nc.compile()
    return nc


def get_nc():
    global _NC_CACHE
    if _NC_CACHE is None:
        _NC_CACHE = _build_nc()
    return _NC_CACHE


def prep_inputs(embeddings, context_feat, W_node, W_fixed, W_step, W_out, action_mask):
    """Host-side layout prep + sharding. Returns per-core in_maps."""
    emb = np.ascontiguousarray(np.asarray(embeddings, dtype=np.float32))
    ctx = np.asarray(context_feat, dtype=np.float32)
    W_node = np.asarray(W_node, dtype=np.float32)
    W_fixed = np.asarray(W_fixed, dtype=np.float32)
    W_step = np.asarray(W_step, dtype=np.float32)
    W_out = np.asarray(W_out, dtype=np.float32)
    mask = np.asarray(action_mask)

    embp = np.zeros((B, NP, D), dtype=np.float32)
    embp[:, :N, :] = emb
    # n-major chunks: e_nd[b, p, 128c + j] = emb[b, 128c + p, j]
    e_nd = np.ascontiguousarray(
        embp.reshape(B, CH, 128, D).transpose(0, 2, 1, 3).reshape(B, 128, NP)
    ).astype(BF16_NP)
    # d-major: e_dn[b, d, n]
    e_dn = np.ascontiguousarray(embp.transpose(0, 2, 1)).astype(BF16_NP)

    def group_major(arr_core):
        # [BC, 128, NP] -> [NG, 128, G, NP] (per-partition contiguous per group)
        return np.ascontiguousarray(
            arr_core.reshape(NG, G, 128, NP).transpose(0, 2, 1, 3)
        )

    feasf = np.zeros((B, NP), dtype=np.float32)
    feasf[:, :N] = mask.astype(np.float32)

    # mean_cols on device are raw sums; fold 1/N and the 1/sqrt(dk) q-scale here
    wfix = np.ascontiguousarray(W_fixed / (N * np.sqrt(DK)))
    wstep = np.ascontiguousarray((W_step / np.sqrt(DK)).reshape(2, 128, 128))
    wgkT = np.ascontiguousarray(W_node[:, :D].T)
    wgv = np.ascontiguousarray(W_node[:, D : 2 * D]).astype(BF16_NP)
    wout = np.ascontiguousarray(W_out).astype(BF16_NP)
    wlkT = np.ascontiguousarray(W_node[:, 2 * D :].T / np.sqrt(D)).astype(BF16_NP)
    hmask = np.zeros((128, H), dtype=np.float32)
    for h in range(H):
        hmask[DK * h : DK * (h + 1), h] = 1.0
    hmask = np.tile(hmask, (1, G))  # [128, (bb, h)]

    in_maps = []
    for i in range(NCORES):
        sl = slice(BC * i, BC * (i + 1))
        f = feasf[sl].reshape(NG, G, CH, 128)          # [g, bb, c, p]
        f_t = f.transpose(0, 3, 1, 2)                   # [g, p, bb, c]
        feas_rep = np.ascontiguousarray(
            np.repeat(f_t[..., None], H, axis=-1).reshape(NG, 128, CF)
        ).astype(BF16_NP)
        binf = np.where(f_t > 0, np.float32(0), np.float32(-np.inf))
        binf = np.ascontiguousarray(binf.reshape(NG, 128, LF))
        ctxT = np.ascontiguousarray(ctx[sl].T.reshape(2, 128, BC))
        in_maps.append(
            {
                "e_nd": group_major(e_nd[sl]),
                "e_dn": group_major(e_dn[sl]),
                "feas": feas_rep,
                "binf": binf,
                "ctxT": ctxT,
                "wfix": wfix,
                "wstep": wstep,
                "wgkT": wgkT,
                "wgv": wgv,
                "wout": wout,
                "wlkT": wlkT,
                "hmask": np.ascontiguousarray(hmask),
            }
        )
    return in_maps


def gather_output(results):
    out = np.empty((B, 1, N), dtype=np.float32)
    for i in range(NCORES):
        o = np.asarray(results[i]["out_dev"], dtype=np.float32)  # [NG, 128, 128]
        o = o.reshape(NG, 128, G, CH).transpose(0, 2, 3, 1).reshape(BC, NP)
        out[BC * i : BC * (i + 1), 0, :] = o[:, :N]
    return out


def kernel(embeddings, context_feat, W_node, W_fixed, W_step, W_out, action_mask,
           **run_kwargs):
    in_maps = prep_inputs(
        embeddings, context_feat, W_node, W_fixed, W_step, W_out, action_mask
    )
    nc = get_nc()
    res = run_bass_kernel_spmd(nc, in_maps, core_ids=list(range(NCORES)), **run_kwargs)
    out = gather_output(res.results)
    if run_kwargs.get("trace"):
        kernel.last_exec_time_ns = res.exec_time_ns
        kernel.last_results = res
    return out


# revision 53
# speedup vs baseline: 1.1480x; 1.1480x over previous
"""Trainium2 Bass kernel for nn_Decoder (sparse_attention, B=512, N=1000, D=128, H=8).

Strategy
--------
Pure data parallel: batch B=512 sharded across 8 NeuronCores (64 per core).

The reference computes, per batch b:
    gc      = mean_n(emb) @ W_fixed                          [D]
    q       = gc + ctx @ W_step                              [D]
    K,V,lk  = heads(emb @ W_node)                            (never materialized here)
    compat  = (q_h . K_h[n]) / sqrt(dk)                      [H,N]
    attn    = softmax(mask(compat))                          [H,N]
    glimpse = concat_h(attn_h @ V_h) @ W_out                 [D]
    logits  = mask(tanh(glimpse . lk[n] / sqrt(D)) * 10)     [N]

We fold the big [D,3D] projection into per-batch vectors so embeddings are the
only large tensor ever touched:
    compat[n,h] = emb[n,:] @ U[:,h]    with  U = W_gk @ blockdiag(q)/sqrt(dk)
    aT[:,h]     = sum_n attn[h,n] emb[n,:]   (attention-weighted embedding sum)
    glimpse     = (W_out.T @ select_heads(W_gv.T @ aT)),  w_l = W_lk @ glimpse/sqrt(D)
    logits[n]   = emb[n,:] @ w_l

Softmax is computed without max-subtraction (compat range is ~[-10, 10] for
these inputs, exp is safe in fp32) and masking is applied multiplicatively
after exp, which is mathematically identical to -inf masking.

Per core, batches are processed in 4 groups of 16 so all tail element-wise ops
run on fully packed [128, *] tiles. Embeddings are supplied in two bf16
layouts (n-major and d-major, host-prepared) because the TensorEngine can only
contract over the partition axis; all PE passes then stream 8-16 column moving
operands against 128x128 embedding chunks loaded as (fast-weight-load) weights.
"""

import numpy as np
import ml_dtypes

import concourse.bass as bass
from concourse import bacc
from concourse.tile import TileContext
from concourse import mybir
from concourse.bass_utils import run_bass_kernel_spmd

F32 = mybir.dt.float32
BF16 = mybir.dt.bfloat16
BF16_NP = ml_dtypes.bfloat16

B, N, D, H = 512, 1000, 128, 8
DK = D // H
NCORES = 8
BC = B // NCORES          # 64 batches per core
NP = 1024                 # padded N
CH = NP // 128            # 8 chunks of 128 nodes
G = 16                    # batches per group
NG = BC // G              # groups per core
CF = G * CH * H           # compat/attn free size per group
LF = G * CH               # logits free size per group
UF = G * H                # (batch, head) packed free size

_NC_CACHE = None


def _build_nc():
    """Build the single-core Bass program (shared SPMD across 8 cores)."""
    nc = bacc.Bacc("TRN2", target_bir_lowering=False)

    # partition-major group layout: [g][p][b within group][n] — per-partition
    # contiguous 32 KB per group for full-rate DMA
    e_nd_d = nc.dram_tensor("e_nd", [NG, 128, G, NP], BF16, kind="ExternalInput")
    e_dn_d = nc.dram_tensor("e_dn", [NG, 128, G, NP], BF16, kind="ExternalInput")
    feas_d = nc.dram_tensor("feas", [NG, 128, CF], BF16, kind="ExternalInput")
    binf_d = nc.dram_tensor("binf", [NG, 128, LF], F32, kind="ExternalInput")
    ctxT_d = nc.dram_tensor("ctxT", [2, 128, BC], F32, kind="ExternalInput")
    wfix_d = nc.dram_tensor("wfix", [128, 128], F32, kind="ExternalInput")
    wstep_d = nc.dram_tensor("wstep", [2, 128, 128], F32, kind="ExternalInput")
    wgkT_d = nc.dram_tensor("wgkT", [128, 128], F32, kind="ExternalInput")
    wgv_d = nc.dram_tensor("wgv", [128, 128], BF16, kind="ExternalInput")
    wout_d = nc.dram_tensor("wout", [128, 128], BF16, kind="ExternalInput")
    wlkT_d = nc.dram_tensor("wlkT", [128, 128], BF16, kind="ExternalInput")
    hmask_d = nc.dram_tensor("hmask", [128, UF], F32, kind="ExternalInput")
    out_d = nc.dram_tensor("out_dev", [NG, 128, LF], F32, kind="ExternalOutput")

    with TileContext(nc) as tc:
        with (
            tc.tile_pool(name="consts", bufs=1) as consts,
            tc.tile_pool(name="emb", bufs=2) as epool,
            tc.tile_pool(name="grp", bufs=2) as gpool,
            tc.tile_pool(name="scr", bufs=2) as spool,
            tc.tile_pool(name="pbig", bufs=2, space="PSUM") as pbig,
            tc.tile_pool(name="pat", bufs=2, space="PSUM") as pat,
            tc.tile_pool(name="pmisc", bufs=2, space="PSUM") as pmisc,
        ):
            # ---- constants ----
            wfix_sb = consts.tile([128, 128], F32)
            nc.sync.dma_start(out=wfix_sb, in_=wfix_d[:, :])
            wstep_sb = consts.tile([128, 2, 128], F32)
            nc.sync.dma_start(out=wstep_sb, in_=wstep_d.rearrange("k p j -> p k j"))
            wgkT_sb = consts.tile([128, 128], F32)
            nc.sync.dma_start(out=wgkT_sb, in_=wgkT_d[:, :])
            wgv_sb = consts.tile([128, 128], BF16)
            nc.sync.dma_start(out=wgv_sb, in_=wgv_d[:, :])
            wout_sb = consts.tile([128, 128], BF16)
            nc.sync.dma_start(out=wout_sb, in_=wout_d[:, :])
            wlkT_sb = consts.tile([128, 128], BF16)
            nc.sync.dma_start(out=wlkT_sb, in_=wlkT_d[:, :])
            hmask_sb = consts.tile([128, UF], F32)
            nc.sync.dma_start(out=hmask_sb, in_=hmask_d[:, :])
            ctxT_sb = consts.tile([128, 2, BC], F32)
            nc.sync.dma_start(out=ctxT_sb, in_=ctxT_d.rearrange("k p b -> p k b"))

            ones_col = consts.tile([128, 1], BF16)
            nc.vector.memset(ones_col, 1.0)
            ones_row = consts.tile([1, 128], F32)
            nc.vector.memset(ones_row, 1.0)

            # step_cols[j, b] = (ctx[b] @ W_step/4)[j]   (one-time)
            step_ps = pmisc.tile([128, BC], F32, tag="misc")
            for k in range(2):
                nc.tensor.matmul(
                    step_ps,
                    lhsT=wstep_sb[:, k, :],
                    rhs=ctxT_sb[:, k, :],
                    start=(k == 0),
                    stop=(k == 1),
                )
            step_cols = consts.tile([128, BC], F32)
            nc.vector.tensor_copy(step_cols, step_ps)

            # Two-stage software-pipelined emission. Engines execute their
            # instruction streams strictly in order, so the emission order IS
            # the schedule: group g+1's head phase (compat/exp) is emitted
            # before group g's tail (aT/logits), and loads/means run 2 and 1
            # groups ahead respectively, so no engine head-of-line-blocks.
            QG = G // 4

            def load_group(g):
                # quarter-group DMAs (1 MB each); e_dn first (needed earliest)
                e_dn_q, e_nd_q = [], []
                for q in range(4):
                    t = epool.tile(
                        [128, QG, NP], BF16, tag="e_dn", name=f"edn{g}_{q}", bufs=12
                    )
                    nc.sync.dma_start(out=t, in_=e_dn_d[g, :, q * QG : (q + 1) * QG, :])
                    e_dn_q.append(t)
                for q in range(4):
                    t = epool.tile(
                        [128, QG, NP], BF16, tag="e_nd", name=f"end{g}_{q}", bufs=8
                    )
                    nc.sync.dma_start(out=t, in_=e_nd_d[g, :, q * QG : (q + 1) * QG, :])
                    e_nd_q.append(t)
                e_dns = [e_dn_q[bb // QG][:, bb % QG, :] for bb in range(G)]
                e_nds = [e_nd_q[bb // QG][:, bb % QG, :] for bb in range(G)]
                return e_dns, e_nds

            def do_means(g, e_dns):
                # raw embedding sums (scale folded into wfix host-side),
                # split ACT/DVE so neither engine bottlenecks
                mean_cols = gpool.tile([128, G], F32, tag="mean", name=f"mc{g}")
                for bb in range(G):
                    if bb % 2 == 0:
                        scr = spool.tile([128, NP], BF16, tag="scr", name=f"scr{g}_{bb}")
                        nc.scalar.activation(
                            out=scr,
                            in_=e_dns[bb],
                            func=mybir.ActivationFunctionType.Copy,
                            bias=0.0,
                            scale=1.0,
                            accum_out=mean_cols[:, bb : bb + 1],
                        )
                    else:
                        nc.vector.tensor_reduce(
                            out=mean_cols[:, bb : bb + 1],
                            in_=e_dns[bb],
                            axis=mybir.AxisListType.X,
                            op=mybir.AluOpType.add,
                        )
                return mean_cols

            def stage1(g, e_dns, mean_cols):
                """q, U, compat, exp, feas-mask -> attn tile for group g."""
                q_ps = pmisc.tile([128, G], F32, tag="misc", name=f"qps{g}")
                nc.tensor.matmul(q_ps, lhsT=wfix_sb, rhs=mean_cols, start=True, stop=True)
                qs = gpool.tile([128, G], F32, tag="qs", name=f"qs{g}")
                nc.vector.tensor_tensor(
                    qs, q_ps, step_cols[:, g * G : (g + 1) * G], mybir.AluOpType.add
                )
                qblk = gpool.tile([128, UF], F32, tag="qblk", name=f"qb{g}")
                nc.vector.scalar_tensor_tensor(
                    out=qblk.rearrange("p (bb h) -> p bb h", bb=G),
                    in0=qs.unsqueeze(2).broadcast_to((128, G, H)),
                    scalar=1.0,
                    in1=hmask_sb.rearrange("p (bb h) -> p bb h", bb=G),
                    op0=mybir.AluOpType.mult,
                    op1=mybir.AluOpType.mult,
                )
                u_ps = pmisc.tile([128, UF], F32, tag="misc", name=f"ups{g}")
                nc.tensor.matmul(u_ps, lhsT=wgkT_sb, rhs=qblk, start=True, stop=True)
                u_sb = gpool.tile([128, UF], BF16, tag="u", name=f"u{g}")
                nc.vector.tensor_copy(u_sb, u_ps)

                # compat[n, (bb,c,h)] = emb_dn_chunk.T @ U[b]
                compat_ps = pbig.tile([128, CF], F32, tag="big", name=f"cp{g}")
                for bb in range(G):
                    for c in range(CH):
                        o = 64 * bb + 8 * c
                        nc.tensor.matmul(
                            compat_ps[:, o : o + 8],
                            lhsT=e_dns[bb][:, 128 * c : 128 * (c + 1)],
                            rhs=u_sb[:, 8 * bb : 8 * bb + 8],
                            start=True,
                            stop=True,
                        )

                # attn = exp(compat) * feas   (no max-sub: |compat| < ~15)
                attn = gpool.tile([128, CF], BF16, tag="attn", name=f"attn{g}")
                nc.scalar.activation(
                    out=attn,
                    in_=compat_ps,
                    func=mybir.ActivationFunctionType.Exp,
                    bias=0.0,
                    scale=1.0,
                )
                feas_sb = gpool.tile([128, CF], BF16, tag="feas", name=f"feas{g}")
                nc.gpsimd.dma_start(out=feas_sb, in_=feas_d[g, :, :])
                nc.vector.tensor_tensor(attn, attn, feas_sb, mybir.AluOpType.mult)
                return attn

            tiles = {0: load_group(0)}
            tiles[1] = load_group(1)
            means = {0: do_means(0, tiles[0][0])}
            attns = {0: stage1(0, tiles[0][0], means.pop(0))}
            means[1] = do_means(1, tiles[1][0])
            for g in range(NG):
                e_dns, e_nds = tiles.pop(g)
                attn = attns.pop(g)
                if g + 2 < NG:
                    tiles[g + 2] = load_group(g + 2)
                if g + 1 < NG:
                    attns[g + 1] = stage1(g + 1, tiles[g + 1][0], means.pop(g + 1))

                # ---- Z (per bb one matmul) and aT (8-chunk accumulation) ----
                z_ps = pbig.tile([1, CF], F32, tag="big", name=f"zp{g}")
                aT_sb = gpool.tile([128, UF], BF16, tag="aT")
                for bb in range(G):
                    nc.tensor.matmul(
                        z_ps[:, 64 * bb : 64 * (bb + 1)],
                        lhsT=ones_col,
                        rhs=attn[:, 64 * bb : 64 * (bb + 1)],
                        start=True,
                        stop=True,
                    )
                    at_ps = pat.tile([128, 8], F32, tag="at", name=f"at{g}_{bb}")
                    for c in range(CH):
                        nc.tensor.matmul(
                            at_ps,
                            lhsT=e_nds[bb][:, 128 * c : 128 * (c + 1)],
                            rhs=attn[:, 64 * bb + 8 * c : 64 * bb + 8 * c + 8],
                            start=(c == 0),
                            stop=(c == CH - 1),
                        )
                    nc.scalar.copy(aT_sb[:, 8 * bb : 8 * bb + 8], at_ps)

                # Z reduce over chunks -> [1, (bb,h)], broadcast via outer product
                z_red = gpool.tile([1, UF], F32, tag="zred")
                nc.vector.tensor_reduce(
                    out=z_red.rearrange("p (bb h) -> p bb h", bb=G),
                    in_=z_ps.rearrange("p (bb c h) -> p bb h c", bb=G, c=CH),
                    axis=mybir.AxisListType.X,
                    op=mybir.AluOpType.add,
                )
                rz = gpool.tile([1, UF], F32, tag="rz")
                nc.vector.reciprocal(rz, z_red)
                zb_ps = pmisc.tile([128, UF], F32, tag="misc", name=f"zb{g}")
                nc.tensor.matmul(zb_ps, lhsT=ones_row, rhs=rz, start=True, stop=True)
                zb_sb = gpool.tile([128, UF], F32, tag="zb")
                nc.vector.tensor_copy(zb_sb, zb_ps)

                # ---- X = W_gv.T-path; normalize by Z; select head blocks ----
                x_ps = pmisc.tile([128, UF], F32, tag="misc", name=f"xps{g}")
                nc.tensor.matmul(x_ps, lhsT=wgv_sb, rhs=aT_sb, start=True, stop=True)
                xn = gpool.tile([128, UF], F32, tag="xn")
                nc.vector.tensor_tensor(xn, x_ps, zb_sb, mybir.AluOpType.mult)
                xh = gpool.tile([128, UF], F32, tag="xh")
                nc.vector.tensor_tensor(xh, xn, hmask_sb, mybir.AluOpType.mult)
                heads = gpool.tile([128, G], BF16, tag="heads")
                with nc.allow_low_precision(reason="8-wide head-select sum, bf16 out"):
                    nc.vector.tensor_reduce(
                        out=heads,
                        in_=xh.rearrange("p (bb h) -> p bb h", bb=G),
                        axis=mybir.AxisListType.X,
                        op=mybir.AluOpType.add,
                    )

                # ---- glimpse, w_l ----
                gl_ps = pmisc.tile([128, G], F32, tag="misc", name=f"gl{g}")
                nc.tensor.matmul(gl_ps, lhsT=wout_sb, rhs=heads, start=True, stop=True)
                gl_sb = gpool.tile([128, G], BF16, tag="gl")
                nc.vector.tensor_copy(gl_sb, gl_ps)
                wl_ps = pmisc.tile([128, G], F32, tag="misc", name=f"wl{g}")
                nc.tensor.matmul(wl_ps, lhsT=wlkT_sb, rhs=gl_sb, start=True, stop=True)
                wl_sb = gpool.tile([128, G], BF16, tag="wl")
                nc.vector.tensor_copy(wl_sb, wl_ps)

                # ---- logits[n-in-chunk, (bb,c)] = emb_dn_chunk.T @ w_l[b] ----
                log_ps = pmisc.tile([128, LF], F32, tag="misc", name=f"lg{g}")
                for bb in range(G):
                    for c in range(CH):
                        nc.tensor.matmul(
                            log_ps[:, 8 * bb + c : 8 * bb + c + 1],
                            lhsT=e_dns[bb][:, 128 * c : 128 * (c + 1)],
                            rhs=wl_sb[:, bb : bb + 1],
                            start=True,
                            stop=True,
                        )

                # ---- tanh, *10, -inf mask, store ----
                th = gpool.tile([128, LF], F32, tag="th")
                nc.scalar.activation(
                    out=th,
                    in_=log_ps,
                    func=mybir.ActivationFunctionType.Tanh,
                    bias=0.0,
                    scale=1.0,
                )
                binf_sb = gpool.tile([128, LF], F32, tag="binf")
                nc.gpsimd.dma_start(out=binf_sb, in_=binf_d[g, :, :])
                out_sb = gpool.tile([128, LF], F32, tag="out")
                nc.vector.scalar_tensor_tensor(
                    out=out_sb,
                    in0=th,
                    scalar=10.0,
                    in1=binf_sb,
                    op0=mybir.AluOpType.mult,
                    op1=mybir.AluOpType.add,
                )
                nc.gpsimd.dma_start(out=out_d[g, :, :], in_=out_sb)

                # mean reductions for group g+2 at the very end of the
                # iteration: ACT/DVE execute them while PE handles the next
                # iteration's tail matmuls
                if g + 2 < NG:
                    means[g + 2] = do_means(g + 2, tiles[g + 2][0])

    nc.compile()
    return nc


def get_nc():
    global _NC_CACHE
    if _NC_CACHE is None:
        _NC_CACHE = _build_nc()
    return _NC_CACHE


def prep_inputs(embeddings, context_feat, W_node, W_fixed, W_step, W_out, action_mask):
    """Host-side layout prep + sharding. Returns per-core in_maps."""
    emb = np.ascontiguousarray(np.asarray(embeddings, dtype=np.float32))
    ctx = np.asarray(context_feat, dtype=np.float32)
    W_node = np.asarray(W_node, dtype=np.float32)
    W_fixed = np.asarray(W_fixed, dtype=np.float32)
    W_step = np.asarray(W_step, dtype=np.float32)
    W_out = np.asarray(W_out, dtype=np.float32)
    mask = np.asarray(action_mask)

    embp = np.zeros((B, NP, D), dtype=np.float32)
    embp[:, :N, :] = emb
    # n-major chunks: e_nd[b, p, 128c + j] = emb[b, 128c + p, j]
    e_nd = np.ascontiguousarray(
        embp.reshape(B, CH, 128, D).transpose(0, 2, 1, 3).reshape(B, 128, NP)
    ).astype(BF16_NP)
    # d-major: e_dn[b, d, n]
    e_dn = np.ascontiguousarray(embp.transpose(0, 2, 1)).astype(BF16_NP)

    def group_major(arr_core):
        # [BC, 128, NP] -> [NG, 128, G, NP] (per-partition contiguous per group)
        return np.ascontiguousarray(
            arr_core.reshape(NG, G, 128, NP).transpose(0, 2, 1, 3)
        )

    feasf = np.zeros((B, NP), dtype=np.float32)
    feasf[:, :N] = mask.astype(np.float32)

    # mean_cols on device are raw sums; fold 1/N and the 1/sqrt(dk) q-scale here
    wfix = np.ascontiguousarray(W_fixed / (N * np.sqrt(DK)))
    wstep = np.ascontiguousarray((W_step / np.sqrt(DK)).reshape(2, 128, 128))
    wgkT = np.ascontiguousarray(W_node[:, :D].T)
    wgv = np.ascontiguousarray(W_node[:, D : 2 * D]).astype(BF16_NP)
    wout = np.ascontiguousarray(W_out).astype(BF16_NP)
    wlkT = np.ascontiguousarray(W_node[:, 2 * D :].T / np.sqrt(D)).astype(BF16_NP)
    hmask = np.zeros((128, H), dtype=np.float32)
    for h in range(H):
        hmask[DK * h : DK * (h + 1), h] = 1.0
    hmask = np.tile(hmask, (1, G))  # [128, (bb, h)]

    in_maps = []
    for i in range(NCORES):
        sl = slice(BC * i, BC * (i + 1))
        f = feasf[sl].reshape(NG, G, CH, 128)          # [g, bb, c, p]
        f_t = f.transpose(0, 3, 1, 2)                   # [g, p, bb, c]
        feas_rep = np.ascontiguousarray(
            np.repeat(f_t[..., None], H, axis=-1).reshape(NG, 128, CF)
        ).astype(BF16_NP)
        binf = np.where(f_t > 0, np.float32(0), np.float32(-np.inf))
        binf = np.ascontiguousarray(binf.reshape(NG, 128, LF))
        ctxT = np.ascontiguousarray(ctx[sl].T.reshape(2, 128, BC))
        in_maps.append(
            {
                "e_nd": group_major(e_nd[sl]),
                "e_dn": group_major(e_dn[sl]),
                "feas": feas_rep,
                "binf": binf,
                "ctxT": ctxT,
                "wfix": wfix,
                "wstep": wstep,
                "wgkT": wgkT,
                "wgv": wgv,
                "wout": wout,
                "wlkT": wlkT,
                "hmask": np.ascontiguousarray(hmask),
            }
        )
    return in_maps


def gather_output(results):
    out = np.empty((B, 1, N), dtype=np.float32)
    for i in range(NCORES):
        o = np.asarray(results[i]["out_dev"], dtype=np.float32)  # [NG, 128, 128]
        o = o.reshape(NG, 128, G, CH).transpose(0, 2, 3, 1).reshape(BC, NP)
        out[BC * i : BC * (i + 1), 0, :] = o[:, :N]
    return out


def kernel(embeddings, context_feat, W_node, W_fixed, W_step, W_out, action_mask,
           **run_kwargs):
    in_maps = prep_inputs(
        embeddings, context_feat, W_node, W_fixed, W_step, W_out, action_mask
    )
    nc = get_nc()
    res = run_bass_kernel_spmd(nc, in_maps, core_ids=list(range(NCORES)), **run_kwargs)
    out = gather_output(res.results)
    if run_kwargs.get("trace"):
        kernel.last_exec_time_ns = res.exec_time_ns
        kernel.last_results = res
    return out
